# revision 47
# baseline (speedup 1.0000x reference)
"""Transformer-XL relative-position multi-head attention on 8 Trainium2 cores.

Sharding: tensor-parallel over heads (2 heads/core) for projections+attention,
then AllToAll to redistribute awv^T head-major -> token-sharded, out-projection
+ residual + LayerNorm token-sharded (512 tokens/core).

Rel-shift: R = (q+v) @ p^T is computed in (i, r) coords and bounced through
DRAM with row stride 2049, appending a zero element per row:
    R'[i*2049 + r] = R[i, r],  R'[i*2049 + 2048] = 0
A single transposed read at partition stride 2048 then reproduces the exact
reference rel-shift semantics for every (i, j):
    shifted[i, j] = R'[2048*i + 1023 + j]
      = R[i, 1023+j-i]        (j-i <= 1024)
      = 0                     (j-i == 1025, the appended zero)
      = R[i+1, j-i-1026]      (j-i >= 1026, the reference reshape wrap)
so scores need no masks/adds: content matmul accumulates + one injection
matmul of the transposed tile per 512-column PSUM bank.

Scores live in (key j, query i) layout so AV needs no transposes: exp is
unnormalized, a ones-column in the AV matmul produces the softmax denominator,
and the division is applied to awv^T (65 x 512 per head) after the fact.

R(bh+1) compute/copy/write is software-pipelined into attn(bh)'s t-loop, and
R(0) into the phase-1 projection loop, to keep PE continuously busy.
"""
import os
import numpy as np
import ml_dtypes

import concourse.bass as bass
import concourse.mybir as mybir
import concourse.tile as tile
from concourse import bacc
from concourse.bass_utils import run_bass_kernel_spmd
from concourse.masks import make_identity
import bass_rust

BF = mybir.dt.bfloat16
F32 = mybir.dt.float32
AF = mybir.ActivationFunctionType
ALU = mybir.AluOpType
bf16 = ml_dtypes.bfloat16

S = 1024
PREV = 1024
T = 2048
B = 4
D = 1024
H = 16
d = 64
NC = 8
SCALE = 1.0 / 8.0
LN_EPS = 1e-5

RROW = 2049                      # R' row stride (2048 scores + 1 zero)
RSZ = 1023 * RROW + 2048 + 1     # per-bh R' slot size = 2098176


def _ap(handle, offset, pattern):
    return bass_rust.AP(tensor=handle, offset=offset, ap=pattern)


def _body(nc, tc, io):
    out_t = io["out"]
    r2 = [io["r2a"], io["r2b"], io["r2c"], io["r2d"]]  # rotation: no false deps
    a2a_in0_t = io["a2a_in0"]
    a2a_in1_t = io["a2a_in1"]
    a2a_out0_t = io["a2a_out0"]
    a2a_out1_t = io["a2a_out1"]

    def rslot(bh):
        return r2[bh % 4], (bh // 4) * RSZ

    with tc.tile_pool(name="res", bufs=1) as res:
        # ---- persistent tiles ----
        kt = res.tile([128, B * T], BF, tag="kt")          # k^T, (2*d, b-major tokens)
        vsb = res.tile([128, 64 * 130], BF, tag="vsb")     # [v_h0|1|v_h1|1] per j-tile
        qu = res.tile([128, B * S], BF, tag="qu")
        qv = res.tile([128, B * S], BF, tag="qv")
        pt = res.tile([128, T], BF, tag="pt")
        wout = res.tile([128, 8 * D], BF, tag="wout")
        wk = res.tile([128, D], BF, tag="wk")
        wv = res.tile([128, D], BF, tag="wv")
        wq = res.tile([128, D], BF, tag="wq")
        wp = res.tile([128, D], BF, tag="wp")
        awvt0 = res.tile([64, B * S], BF, tag="awvt0")
        awvt1 = res.tile([64, B * S], BF, tag="awvt1")
        awvf = res.tile([128, 8 * 512], BF, tag="awvf")    # gathered awv^T K-tiles
        ident = res.tile([128, 128], BF, tag="ident")
        zcol = res.tile([128, 8], BF, tag="zcol")          # 1024 zeros for R' pad col
        ub = res.tile([128, 1], F32, tag="ub")
        vb = res.tile([128, 1], F32, tag="vb")
        lng_r = res.tile([1, D], F32, tag="lngr")
        lnb_r = res.tile([1, D], F32, tag="lnbr")
        lng_b = res.tile([128, D], F32, tag="lngb")
        lnb_b = res.tile([128, D], F32, tag="lnbb")

        # ---- constant loads ----
        # wp first: the p-projection is the first PE work
        nc.sync.dma_start(wp[:], _ap(io["wp"], 0, [[1024, 128], [1, 1024]]))
        make_identity(nc, ident[:])
        nc.gpsimd.memset(vsb[:], 1.0)
        nc.gpsimd.memset(zcol[:], 0.0)

        with tc.tile_pool(name="psr", bufs=2, space="PSUM") as psr, \
             tc.tile_pool(name="rsp", bufs=6) as rsp:

            # ---------- R(bh) chunk machinery (32 chunks per bh) ----------
            rstate = {}
            pending_writes = []  # (emit_after_chunk_count, closure)
            chunk_clock = [0]

            def flush_writes(slack=int(os.environ.get("K_SLACK", "10"))):
                """Emit deferred R' writes whose source copies finished >= slack
                chunk-slots ago, so the sync queue never head-of-line blocks."""
                while pending_writes and \
                        pending_writes[0][0] + slack <= chunk_clock[0]:
                    pending_writes.pop(0)[1]()

            def r_chunk(bh, c):
                """Emit chunk c (it=c//4, rt=c%4) of R'(bh)."""
                b_, hl_ = bh // 2, bh % 2
                hs_ = slice(hl_ * 64, (hl_ + 1) * 64)
                it, rt = c // 4, c % 4
                if rt == 0:
                    rstate["rs"] = rsp.tile([128, T], BF, tag="rs", name="rs")
                rs = rstate["rs"]
                pr = psr.tile([128, 512], F32, tag="rc")
                nc.tensor.matmul(
                    pr[:],
                    lhsT=qv[hs_, b_ * S + it * 128: b_ * S + (it + 1) * 128],
                    rhs=pt[hs_, rt * 512:(rt + 1) * 512],
                    start=True, stop=True)
                if rt == 0:
                    nc.scalar.activation(rs[:, rt * 512:(rt + 1) * 512], pr[:], AF.Copy)
                else:
                    nc.vector.tensor_copy(rs[:, rt * 512:(rt + 1) * 512], pr[:])
                chunk_clock[0] += 1
                if rt == 3:
                    buf, base = rslot(bh)
                    off = base + it * 128 * RROW

                    def wr(rs=rs, buf=buf, off=off):
                        nc.sync.dma_start(
                            _ap(buf, off, [[RROW, 128], [1, 1024]]),
                            rs[:, 0:1024])
                        nc.sync.dma_start(
                            _ap(buf, off + 1024, [[RROW, 128], [1, 1024]]),
                            rs[:, 1024:2048])
                    pending_writes.append((chunk_clock[0], wr))
                flush_writes()

            # ---------- phase 1: projections (R(0) interleaved) ----------
            with tc.tile_pool(name="xt", bufs=3) as xtp, \
                 tc.tile_pool(name="ps1", bufs=2, space="PSUM") as ps1, \
                 tc.tile_pool(name="psv", bufs=2, space="PSUM") as psv, \
                 tc.tile_pool(name="psT", bufs=2, space="PSUM") as psT, \
                 tc.tile_pool(name="vts", bufs=2) as vtsp:
                # p^T first (R(0) needs it); all 4 loads issued up front
                ptiles = []
                for rt in range(4):
                    ptile = xtp.tile([128, 8 * 512], BF, tag="pt", name="ptile")
                    nc.sync.dma_start(
                        ptile[:],
                        _ap(io["pt"], rt * 512, [[T, 128], [T * 128, 8], [1, 512]]))
                    ptiles.append(ptile)
                for wt_sb, wt_h in ((wk, io["wk"]), (wv, io["wv"]), (wq, io["wq"])):
                    nc.sync.dma_start(
                        wt_sb[:], _ap(wt_h, 0, [[1024, 128], [1, 1024]]))
                nc.sync.dma_start(ub[:], io["ub"][:])
                nc.sync.dma_start(vb[:], io["vb"][:])
                for rt in range(4):
                    ptile = ptiles[rt]
                    pp = ps1.tile([128, 512], F32, tag="mm")
                    for kd in range(8):
                        nc.tensor.matmul(
                            pp[:], lhsT=wp[:, kd * 128:(kd + 1) * 128],
                            rhs=ptile[:, kd * 512:(kd + 1) * 512],
                            start=(kd == 0), stop=(kd == 7))
                    nc.vector.tensor_copy(pt[:, rt * 512:(rt + 1) * 512], pp[:])

                # deferred constant loads (needed later than the projections)
                for buf in r2:
                    for s_ in range(2):
                        nc.sync.dma_start(
                            _ap(buf, s_ * RSZ + 2048, [[RROW, 1024], [1, 1]]),
                            zcol[:])
                nc.sync.dma_start(
                    wout[:], _ap(io["wout"], 0, [[1024, 128], [131072, 8], [1, 1024]]))
                nc.sync.dma_start(lng_r[:], io["lng"][:])
                nc.sync.dma_start(lnb_r[:], io["lnb"][:])

                # token slices; b0's q-slices first so R(0)/R(1) start early
                nt_order = [2, 3, 0, 1] + list(range(4, 16))
                r0_done = 0  # chunks of R(0) followed by R(1): 64 total
                for step, nt in enumerate(nt_order):
                    xtile = xtp.tile([128, 8 * 512], BF, tag="xt")
                    nc.gpsimd.dma_start(
                        xtile[:],
                        _ap(io["xt"], nt * 512,
                            [[B * T, 128], [B * T * 128, 8], [1, 512]]))
                    # k^T
                    ps = ps1.tile([128, 512], F32, tag="mm")
                    for kd in range(8):
                        nc.tensor.matmul(
                            ps[:], lhsT=wk[:, kd * 128:(kd + 1) * 128],
                            rhs=xtile[:, kd * 512:(kd + 1) * 512],
                            start=(kd == 0), stop=(kd == 7))
                    nc.scalar.activation(kt[:, nt * 512:(nt + 1) * 512], ps[:], AF.Copy)
                    # v^T then PE-transpose to token-partition layout
                    pv = psv.tile([128, 512], F32, tag="v")
                    for kd in range(8):
                        nc.tensor.matmul(
                            pv[:], lhsT=wv[:, kd * 128:(kd + 1) * 128],
                            rhs=xtile[:, kd * 512:(kd + 1) * 512],
                            start=(kd == 0), stop=(kd == 7))
                    vt = vtsp.tile([128, 512], BF, tag="vts")
                    nc.vector.tensor_copy(vt[:], pv[:])
                    pT = psT.tile([128, 512], BF, tag="T")
                    for sub in range(4):
                        nc.tensor.transpose(
                            pT[:, sub * 128:(sub + 1) * 128],
                            vt[:, sub * 128:(sub + 1) * 128], ident[:])
                        g = nt * 4 + sub
                        eng = nc.scalar if sub % 2 == 0 else nc.vector
                        sl = pT[:, sub * 128: sub * 128 + 64]
                        sh = pT[:, sub * 128 + 64: sub * 128 + 128]
                        if eng is nc.scalar:
                            nc.scalar.activation(vsb[:, g * 130: g * 130 + 64], sl, AF.Copy)
                            nc.scalar.activation(vsb[:, g * 130 + 65: g * 130 + 129], sh, AF.Copy)
                        else:
                            nc.vector.tensor_copy(vsb[:, g * 130: g * 130 + 64], sl)
                            nc.vector.tensor_copy(vsb[:, g * 130 + 65: g * 130 + 129], sh)
                    # q (input_ tokens only: last 1024 of each batch's 2048)
                    if nt % 4 >= 2:
                        pq = ps1.tile([128, 512], F32, tag="mm")
                        for kd in range(8):
                            nc.tensor.matmul(
                                pq[:], lhsT=wq[:, kd * 128:(kd + 1) * 128],
                                rhs=xtile[:, kd * 512:(kd + 1) * 512],
                                start=(kd == 0), stop=(kd == 7))
                        qc = (nt // 4) * 1024 + (nt % 4 - 2) * 512
                        nc.scalar.activation(qu[:, qc:qc + 512], pq[:], AF.Identity,
                                             bias=ub[:])
                        nc.scalar.activation(qv[:, qc:qc + 512], pq[:], AF.Identity,
                                             bias=vb[:])
                    # interleave R(0)+R(1) chunks once qv(b0)+pt are available
                    if step >= 2:
                        want = min(64, ((step - 1) * 64) // 12)
                        while r0_done < want:
                            r_chunk(r0_done // 32, r0_done % 32)
                            r0_done += 1
                while r0_done < 64:
                    r_chunk(r0_done // 32, r0_done % 32)
                    r0_done += 1
                flush_writes(slack=0)

            # ---------- phases 2+3: attention, R(bh+1) interleaved ----------
            PF = int(os.environ.get("K_PF", "3"))  # d1 prefetch depth (t-steps)
            with tc.tile_pool(name="ps2", bufs=4, space="PSUM") as ps2, \
                 tc.tile_pool(name="psav", bufs=2, space="PSUM") as psav, \
                 tc.tile_pool(name="d1p", bufs=int(os.environ.get("K_D1B", "8"))) as d1p, \
                 tc.tile_pool(name="attn", bufs=8) as atp, \
                 tc.tile_pool(name="nrm", bufs=2) as nrm:
                d1store = {bh_: [None] * 16 for bh_ in range(8)}

                def d1_load(bh_, t):
                    buf_, base_ = rslot(bh_)
                    d1 = d1p.tile([128, 1024], BF, tag="d1", name="d1")
                    q = nc.sync if (os.environ.get("K_D1Q", "1") == "0"
                                    or t % 2 == 0) else nc.scalar
                    q.dma_start_transpose(
                        d1[:],
                        _ap(buf_, base_ + 1023 + 128 * t, [[2048, 1024], [1, 128]]))
                    d1store[bh_][t] = d1

                for t in range(PF):
                    d1_load(0, t)
                for bh in range(8):
                    b, hl = bh // 2, bh % 2
                    hs = slice(hl * 64, (hl + 1) * 64)
                    d1s = d1store[bh]
                    ats = [None] * 16
                    pavs = [psav.tile([65, 512], F32, tag="av", name="pav")
                            for _ in range(2)]
                    rnext_done = 0
                    for t in range(16):
                        if t + PF < 16:
                            d1_load(bh, t + PF)
                        elif bh + 1 < 8:
                            d1_load(bh + 1, t + PF - 16)
                        g = b * 16 + t
                        ats[t] = []
                        for i0h in range(2):
                            sc = ps2.tile([128, 512], F32, tag="sc", name="sc")
                            nc.tensor.matmul(
                                sc[:],
                                lhsT=kt[hs, b * T + t * 128: b * T + (t + 1) * 128],
                                rhs=qu[hs, b * S + i0h * 512: b * S + (i0h + 1) * 512],
                                start=True, stop=False)
                            nc.tensor.matmul(
                                sc[:],
                                lhsT=ident[:],
                                rhs=d1s[t][:, i0h * 512:(i0h + 1) * 512],
                                start=False, stop=True)
                            at = atp.tile([128, 512], BF, tag="at", name="at")
                            nc.scalar.activation(at[:], sc[:], AF.Exp, scale=SCALE)
                            ats[t].append(at)
                        # AV for t-2 (software pipelined so PE never waits on exp)
                        if t > 1:
                            gav = b * 16 + t - 2
                            for i0h in range(2):
                                nc.tensor.matmul(
                                    pavs[i0h][:],
                                    lhsT=vsb[:, gav * 130 + hl * 65:
                                             gav * 130 + hl * 65 + 65],
                                    rhs=ats[t - 2][i0h][:],
                                    start=(t == 2), stop=False)
                            ats[t - 2] = None
                        # interleave R(bh+2): all 32 chunks across t-steps 0..11
                        if bh + 2 < 8:
                            want = min(32, (32 * (t + 1)) // int(os.environ.get("K_SPREAD", "11")))
                            while rnext_done < want:
                                r_chunk(bh + 2, rnext_done)
                                rnext_done += 1
                    flush_writes(slack=0)
                    for tl in (14, 15):
                        for i0h in range(2):
                            nc.tensor.matmul(
                                pavs[i0h][:],
                                lhsT=vsb[:, (b * 16 + tl) * 130 + hl * 65:
                                         (b * 16 + tl) * 130 + hl * 65 + 65],
                                rhs=ats[tl][i0h][:],
                                start=False, stop=(tl == 15))
                    # normalize: row 64 of psav = sum(exp); divide awv rows 0-63
                    awvt = awvt0 if hl == 0 else awvt1
                    for i0h in range(2):
                        awvu = nrm.tile([65, 512], BF, tag="awvu")
                        nc.scalar.activation(awvu[:], pavs[i0h][:], AF.Copy)
                        rec = nrm.tile([1, 512], F32, tag="rec")
                        nc.vector.reciprocal(rec[:], awvu[64:65, :])
                        recb = nrm.tile([64, 512], F32, tag="recb")
                        nc.gpsimd.partition_broadcast(recb[:], rec[:])
                        nc.vector.tensor_tensor(
                            out=awvt[:, b * S + i0h * 512: b * S + (i0h + 1) * 512],
                            in0=awvu[0:64, :], in1=recb[:], op=ALU.mult)
                    # stage this b-slice of awv^T into the A2A input buffer
                    a2a_in_h = a2a_in0_t if hl == 0 else a2a_in1_t
                    nc.sync.dma_start(
                        _ap(a2a_in_h, 2 * b * 32768,
                            [[512, 64], [32768, 2], [1, 512]]),
                        awvt[:, b * S: (b + 1) * S])
                    if bh == 6:
                        # awvt0 fully staged: overlap its AllToAll with attn(7)
                        if io.get("no_cc"):
                            nc.sync.dma_start(a2a_out0_t[:], a2a_in0_t[:])
                        else:
                            nc.gpsimd.collective_compute(
                                "AllToAll", ALU.bypass,
                                replica_groups=[list(range(NC))],
                                ins=[a2a_in0_t[:]], outs=[a2a_out0_t[:]],
                            )

        # ---- phase 4: A2A (second half), out-projection, residual, LayerNorm ----
        if io.get("no_cc"):
            nc.sync.dma_start(a2a_out1_t[:], a2a_in1_t[:])
        else:
            nc.gpsimd.collective_compute(
                "AllToAll", ALU.bypass,
                replica_groups=[list(range(NC))],
                ins=[a2a_in1_t[:]], outs=[a2a_out1_t[:]],
            )
        for c_ in range(8):
            nc.sync.dma_start(
                awvf[0:64, c_ * 512:(c_ + 1) * 512],
                _ap(a2a_out0_t, c_ * 32768, [[512, 64], [1, 512]]))
            nc.sync.dma_start(
                awvf[64:128, c_ * 512:(c_ + 1) * 512],
                _ap(a2a_out1_t, c_ * 32768, [[512, 64], [1, 512]]))
        nc.gpsimd.partition_broadcast(lng_b[:], lng_r[:])
        nc.gpsimd.partition_broadcast(lnb_b[:], lnb_r[:])

        with tc.tile_pool(name="outp", bufs=2) as op_, \
             tc.tile_pool(name="stat", bufs=2) as stp, \
             tc.tile_pool(name="ps3", bufs=2, space="PSUM") as ps3:
            for tt in range(4):
                resid = op_.tile([128, D], F32, tag="resid")
                nc.sync.dma_start(resid[:], io["resid"][tt * 128:(tt + 1) * 128, :])
                o = op_.tile([128, D], F32, tag="o")
                for n2 in range(2):
                    po = ps3.tile([128, 512], F32, tag="mm")
                    for kd in range(8):
                        nc.tensor.matmul(
                            po[:],
                            lhsT=awvf[:, kd * 512 + tt * 128: kd * 512 + (tt + 1) * 128],
                            rhs=wout[:, kd * D + n2 * 512: kd * D + n2 * 512 + 512],
                            start=(kd == 0), stop=(kd == 7))
                    nc.vector.tensor_add(
                        o[:, n2 * 512:(n2 + 1) * 512], po[:],
                        resid[:, n2 * 512:(n2 + 1) * 512])
                # LayerNorm over D
                sm = stp.tile([128, 1], F32, tag="sm")
                nc.vector.tensor_reduce(sm[:], o[:], axis=mybir.AxisListType.X,
                                        op=ALU.add)
                mean = stp.tile([128, 1], F32, tag="mean")
                nc.vector.tensor_scalar_mul(mean[:], sm[:], 1.0 / D)
                cent = op_.tile([128, D], F32, tag="cent")
                nc.vector.tensor_scalar(out=cent[:], in0=o[:], scalar1=mean[:],
                                        scalar2=None, op0=ALU.subtract)
                sq = op_.tile([128, D], F32, tag="sq")
                ssq = stp.tile([128, 1], F32, tag="ssq")
                nc.scalar.activation(sq[:], cent[:], AF.Square, accum_out=ssq[:])
                veps = stp.tile([128, 1], F32, tag="veps")
                nc.vector.tensor_scalar(out=veps[:], in0=ssq[:], scalar1=1.0 / D,
                                        scalar2=LN_EPS, op0=ALU.mult, op1=ALU.add)
                std = stp.tile([128, 1], F32, tag="std")
                nc.scalar.activation(std[:], veps[:], AF.Sqrt)
                rstd = stp.tile([128, 1], F32, tag="rstd")
                nc.vector.reciprocal(rstd[:], std[:])
                y1 = op_.tile([128, D], F32, tag="o")
                nc.vector.scalar_tensor_tensor(
                    out=y1[:], in0=cent[:], scalar=rstd[:], in1=lng_b[:],
                    op0=ALU.mult, op1=ALU.mult)
                yf = op_.tile([128, D], F32, tag="cent")
                nc.vector.tensor_add(yf[:], y1[:], lnb_b[:])
                nc.sync.dma_start(out_t[tt * 128:(tt + 1) * 128, :], yf[:])


_compiled = None


def _build(no_cc=False):
    nc = bacc.Bacc("TRN2", target_bir_lowering=False, debug=False, num_devices=NC)
    io = {}
    io["xt"] = nc.dram_tensor("xt", [D, B * T], BF, kind="ExternalInput")
    io["pt"] = nc.dram_tensor("pt", [D, T], BF, kind="ExternalInput")
    io["wk"] = nc.dram_tensor("wk", [128, D], BF, kind="ExternalInput")
    io["wv"] = nc.dram_tensor("wv", [128, D], BF, kind="ExternalInput")
    io["wq"] = nc.dram_tensor("wq", [128, D], BF, kind="ExternalInput")
    io["wp"] = nc.dram_tensor("wp", [128, D], BF, kind="ExternalInput")
    io["wout"] = nc.dram_tensor("wout", [H * d, D], BF, kind="ExternalInput")
    io["ub"] = nc.dram_tensor("ub", [128, 1], F32, kind="ExternalInput").ap()
    io["vb"] = nc.dram_tensor("vb", [128, 1], F32, kind="ExternalInput").ap()
    io["lng"] = nc.dram_tensor("lng", [1, D], F32, kind="ExternalInput").ap()
    io["lnb"] = nc.dram_tensor("lnb", [1, D], F32, kind="ExternalInput").ap()
    io["resid"] = nc.dram_tensor("resid", [512, D], F32, kind="ExternalInput").ap()
    io["out"] = nc.dram_tensor("out", [512, D], F32, kind="ExternalOutput").ap()
    for nm in ("r2a", "r2b", "r2c", "r2d"):
        io[nm] = nc.dram_tensor(nm, [2, RSZ], BF)
    for nm in ("a2a_in0", "a2a_in1", "a2a_out0", "a2a_out1"):
        io[nm] = nc.dram_tensor(nm, [NC, 64, 512], BF)
    io["no_cc"] = no_cc
    with tile.TileContext(nc) as tc:
        _body(nc, tc, io)
    nc.compile()
    return nc


def _shard(inputs):
    x = np.asarray(inputs["input_"], np.float32)
    pos = np.asarray(inputs["pos_embs"], np.float32)
    mem = np.asarray(inputs["memory"], np.float32)
    u = np.asarray(inputs["u"], np.float32).reshape(-1)
    v = np.asarray(inputs["v"], np.float32).reshape(-1)
    W_kv = np.asarray(inputs["W_kv"], np.float32)
    W_q = np.asarray(inputs["W_q"], np.float32)
    W_p = np.asarray(inputs["W_p"], np.float32)
    W_out = np.asarray(inputs["W_out"], np.float32)
    lng = np.asarray(inputs["ln_g"], np.float32).reshape(1, D)
    lnb = np.asarray(inputs["ln_b"], np.float32).reshape(1, D)

    x_mem = np.concatenate([mem, x], axis=0)                  # (T, B, D)
    xt = np.ascontiguousarray(
        x_mem.transpose(2, 1, 0).reshape(D, B * T)            # (D, b-major tokens)
    ).astype(bf16)
    pt = np.ascontiguousarray(pos.T).astype(bf16)             # (D, T)
    wout_b = W_out.astype(bf16)

    in_maps = []
    for c in range(NC):
        hs = slice(c * 128, (c + 1) * 128)
        b, i0 = c // 2, (c % 2) * 512
        in_maps.append({
            "xt": xt,
            "pt": pt,
            "wk": _wswiz(W_kv[:, hs]),
            "wv": _wswiz(W_kv[:, H * d + c * 128: H * d + (c + 1) * 128]),
            "wq": _wswiz(W_q[:, hs]),
            "wp": _wswiz(W_p[:, hs]),
            "wout": wout_b,
            "ub": np.ascontiguousarray(u[hs].reshape(128, 1)),
            "vb": np.ascontiguousarray(v[hs].reshape(128, 1)),
            "lng": lng,
            "lnb": lnb,
            "resid": np.ascontiguousarray(x[i0:i0 + 512, b, :]),
        })
    return in_maps


def _wswiz(w):
    """(1024, 128) -> (128, 1024) so wt_sb[p, kd*128+j] = w[p+128*kd, j] loads
    as 128 contiguous 2 KiB rows."""
    return np.ascontiguousarray(
        w.reshape(8, 128, 128).transpose(1, 0, 2).reshape(128, 1024)).astype(bf16)


LAST_RESULTS = None


def kernel(**inputs):
    global _compiled, LAST_RESULTS
    if _compiled is None:
        _compiled = _build()
    nc = _compiled
    in_maps = _shard(inputs)
    res = run_bass_kernel_spmd(nc, in_maps, core_ids=list(range(NC)))
    LAST_RESULTS = res
    out = np.empty((S, B, D), np.float32)
    for c in range(NC):
        b, i0 = c // 2, (c % 2) * 512
        out[i0:i0 + 512, b, :] = res.results[c]["out"]
    return out


# revision 49
# speedup vs baseline: 1.9535x; 1.9535x over previous
"""Transformer-XL relative-position multi-head attention on 8 Trainium2 cores.

Sharding: tensor-parallel over heads (2 heads/core) for projections+attention,
then AllToAll to redistribute awv^T head-major -> token-sharded, out-projection
+ residual + LayerNorm token-sharded (512 tokens/core).

Rel-shift: R = (q+v) @ p^T is computed in (i, r) coords and bounced through
DRAM with row stride 2049, appending a zero element per row:
    R'[i*2049 + r] = R[i, r],  R'[i*2049 + 2048] = 0
A single transposed read at partition stride 2048 then reproduces the exact
reference rel-shift semantics for every (i, j):
    shifted[i, j] = R'[2048*i + 1023 + j]
      = R[i, 1023+j-i]        (j-i <= 1024)
      = 0                     (j-i == 1025, the appended zero)
      = R[i+1, j-i-1026]      (j-i >= 1026, the reference reshape wrap)
so scores need no masks/adds: content matmul accumulates + one injection
matmul of the transposed tile per 512-column PSUM bank.

Scores live in (key j, query i) layout so AV needs no transposes: exp is
unnormalized, a ones-column in the AV matmul produces the softmax denominator,
and the division is applied to awv^T (65 x 512 per head) after the fact.

R(bh+1) compute/copy/write is software-pipelined into attn(bh)'s t-loop, and
R(0) into the phase-1 projection loop, to keep PE continuously busy.
"""
import os
import numpy as np
import ml_dtypes

import concourse.bass as bass
import concourse.mybir as mybir
import concourse.tile as tile
from concourse import bacc
from concourse.bass_utils import run_bass_kernel_spmd
from concourse.masks import make_identity
import bass_rust

BF = mybir.dt.bfloat16
F32 = mybir.dt.float32
AF = mybir.ActivationFunctionType
ALU = mybir.AluOpType
bf16 = ml_dtypes.bfloat16

S = 1024
PREV = 1024
T = 2048
B = 4
D = 1024
H = 16
d = 64
NC = 8
SCALE = 1.0 / 8.0
LN_EPS = 1e-5

RROW = 2049                      # R' row stride (2048 scores + 1 zero)
RSZ = 1023 * RROW + 2048 + 1     # per-bh R' slot size = 2098176


def _ap(handle, offset, pattern):
    return bass_rust.AP(tensor=handle, offset=offset, ap=pattern)


def _body(nc, tc, io):
    out_t = io["out"]
    r2 = [io["r2a"], io["r2b"], io["r2c"], io["r2d"]]  # rotation: no false deps
    a2a_in0_t = io["a2a_in0"]
    a2a_in1_t = io["a2a_in1"]
    a2a_out0_t = io["a2a_out0"]
    a2a_out1_t = io["a2a_out1"]

    def rslot(bh):
        return r2[bh % 4], (bh // 4) * RSZ

    with tc.tile_pool(name="res", bufs=1) as res:
        # ---- persistent tiles ----
        kt = res.tile([128, B * T], BF, tag="kt")          # k^T, (2*d, b-major tokens)
        vsb = res.tile([128, 64 * 130], BF, tag="vsb")     # [v_h0|1|v_h1|1] per j-tile
        qu = res.tile([128, B * S], BF, tag="qu")
        qv = res.tile([128, B * S], BF, tag="qv")
        pt = res.tile([128, T], BF, tag="pt")
        wout = res.tile([128, 8 * D], BF, tag="wout")
        wk = res.tile([128, D], BF, tag="wk")
        wv = res.tile([128, D], BF, tag="wv")
        wq = res.tile([128, D], BF, tag="wq")
        wp = res.tile([128, D], BF, tag="wp")
        awvt0 = res.tile([64, B * S], BF, tag="awvt0")
        awvt1 = res.tile([64, B * S], BF, tag="awvt1")
        awvf = res.tile([128, 8 * 512], BF, tag="awvf")    # gathered awv^T K-tiles
        ident = res.tile([128, 128], BF, tag="ident")
        zcol = res.tile([128, 8], BF, tag="zcol")          # 1024 zeros for R' pad col
        ub = res.tile([128, 1], F32, tag="ub")
        vb = res.tile([128, 1], F32, tag="vb")
        lng_r = res.tile([1, D], F32, tag="lngr")
        lnb_r = res.tile([1, D], F32, tag="lnbr")
        lng_b = res.tile([128, D], F32, tag="lngb")
        lnb_b = res.tile([128, D], F32, tag="lnbb")

        # ---- constant loads ----
        # wp first: the p-projection is the first PE work
        nc.sync.dma_start(wp[:], _ap(io["wp"], 0, [[1024, 128], [1, 1024]]))
        make_identity(nc, ident[:])
        nc.gpsimd.memset(vsb[:], 1.0)
        nc.gpsimd.memset(zcol[:], 0.0)

        with tc.tile_pool(name="psr", bufs=2, space="PSUM") as psr, \
             tc.tile_pool(name="rsp", bufs=6) as rsp:

            # ---------- R(bh) chunk machinery (32 chunks per bh) ----------
            rstate = {}
            pending_writes = []  # (emit_after_chunk_count, closure)
            chunk_clock = [0]

            def flush_writes(slack=int(os.environ.get("K_SLACK", "10"))):
                """Emit deferred R' writes whose source copies finished >= slack
                chunk-slots ago, so the sync queue never head-of-line blocks."""
                while pending_writes and \
                        pending_writes[0][0] + slack <= chunk_clock[0]:
                    pending_writes.pop(0)[1]()

            def r_chunk(bh, c):
                """Emit chunk c (it=c//4, rt=c%4) of R'(bh)."""
                b_, hl_ = bh // 2, bh % 2
                hs_ = slice(hl_ * 64, (hl_ + 1) * 64)
                it, rt = c // 4, c % 4
                if rt == 0:
                    rstate["rs"] = rsp.tile([128, T], BF, tag="rs", name="rs")
                rs = rstate["rs"]
                pr = psr.tile([128, 512], F32, tag="rc")
                nc.tensor.matmul(
                    pr[:],
                    lhsT=qv[hs_, b_ * S + it * 128: b_ * S + (it + 1) * 128],
                    rhs=pt[hs_, rt * 512:(rt + 1) * 512],
                    start=True, stop=True)
                if rt == 0:
                    nc.scalar.activation(rs[:, rt * 512:(rt + 1) * 512], pr[:], AF.Copy)
                else:
                    nc.vector.tensor_copy(rs[:, rt * 512:(rt + 1) * 512], pr[:])
                chunk_clock[0] += 1
                if rt == 3:
                    buf, base = rslot(bh)
                    off = base + it * 128 * RROW

                    def wr(rs=rs, buf=buf, off=off):
                        nc.sync.dma_start(
                            _ap(buf, off, [[RROW, 128], [1, 1024]]),
                            rs[:, 0:1024])
                        nc.sync.dma_start(
                            _ap(buf, off + 1024, [[RROW, 128], [1, 1024]]),
                            rs[:, 1024:2048])
                    pending_writes.append((chunk_clock[0], wr))
                flush_writes()

            # ---------- phase 1: projections (R(0) interleaved) ----------
            with tc.tile_pool(name="xt", bufs=3) as xtp, \
                 tc.tile_pool(name="ps1", bufs=2, space="PSUM") as ps1, \
                 tc.tile_pool(name="psv", bufs=2, space="PSUM") as psv, \
                 tc.tile_pool(name="psT", bufs=2, space="PSUM") as psT, \
                 tc.tile_pool(name="vts", bufs=2) as vtsp:
                # p^T first (R(0) needs it); all 4 loads issued up front
                ptiles = []
                for rt in range(4):
                    ptile = xtp.tile([128, 8 * 512], BF, tag="pt", name="ptile")
                    nc.sync.dma_start(
                        ptile[:],
                        _ap(io["pt"], rt * 512, [[T, 128], [T * 128, 8], [1, 512]]))
                    ptiles.append(ptile)
                for wt_sb, wt_h in ((wk, io["wk"]), (wv, io["wv"]), (wq, io["wq"])):
                    nc.sync.dma_start(
                        wt_sb[:], _ap(wt_h, 0, [[1024, 128], [1, 1024]]))
                nc.sync.dma_start(ub[:], io["ub"][:])
                nc.sync.dma_start(vb[:], io["vb"][:])
                xpre = {}
                for nt in (2, 3):
                    xtile = xtp.tile([128, 8 * 512], BF, tag="xt", name="xt")
                    nc.gpsimd.dma_start(
                        xtile[:],
                        _ap(io["xt"], nt * 512,
                            [[B * T, 128], [B * T * 128, 8], [1, 512]]))
                    xpre[nt] = xtile
                for rt in range(4):
                    ptile = ptiles[rt]
                    pp = ps1.tile([128, 512], F32, tag="mm")
                    for kd in range(8):
                        nc.tensor.matmul(
                            pp[:], lhsT=wp[:, kd * 128:(kd + 1) * 128],
                            rhs=ptile[:, kd * 512:(kd + 1) * 512],
                            start=(kd == 0), stop=(kd == 7))
                    nc.vector.tensor_copy(pt[:, rt * 512:(rt + 1) * 512], pp[:])

                # deferred constant loads (needed later than the projections)
                for buf in r2:
                    for s_ in range(2):
                        nc.sync.dma_start(
                            _ap(buf, s_ * RSZ + 2048, [[RROW, 1024], [1, 1]]),
                            zcol[:])
                nc.sync.dma_start(
                    wout[:], _ap(io["wout"], 0, [[1024, 128], [131072, 8], [1, 1024]]))
                nc.sync.dma_start(lng_r[:], io["lng"][:])
                nc.sync.dma_start(lnb_r[:], io["lnb"][:])

                # token slices; b0's q-slices first so R(0)/R(1) start early
                nt_order = [2, 3, 0, 1] + list(range(4, 16))
                r0_done = 0  # chunks of R(0) followed by R(1): 64 total
                for step, nt in enumerate(nt_order):
                    if nt in xpre:
                        xtile = xpre.pop(nt)
                    else:
                        xtile = xtp.tile([128, 8 * 512], BF, tag="xt", name="xt")
                        nc.gpsimd.dma_start(
                            xtile[:],
                            _ap(io["xt"], nt * 512,
                                [[B * T, 128], [B * T * 128, 8], [1, 512]]))
                    # k^T
                    ps = ps1.tile([128, 512], F32, tag="mm")
                    for kd in range(8):
                        nc.tensor.matmul(
                            ps[:], lhsT=wk[:, kd * 128:(kd + 1) * 128],
                            rhs=xtile[:, kd * 512:(kd + 1) * 512],
                            start=(kd == 0), stop=(kd == 7))
                    nc.scalar.activation(kt[:, nt * 512:(nt + 1) * 512], ps[:], AF.Copy)
                    # v^T then PE-transpose to token-partition layout
                    pv = psv.tile([128, 512], F32, tag="v")
                    for kd in range(8):
                        nc.tensor.matmul(
                            pv[:], lhsT=wv[:, kd * 128:(kd + 1) * 128],
                            rhs=xtile[:, kd * 512:(kd + 1) * 512],
                            start=(kd == 0), stop=(kd == 7))
                    vt = vtsp.tile([128, 512], BF, tag="vts")
                    nc.vector.tensor_copy(vt[:], pv[:])
                    pT = psT.tile([128, 512], BF, tag="T")
                    for sub in range(4):
                        nc.tensor.transpose(
                            pT[:, sub * 128:(sub + 1) * 128],
                            vt[:, sub * 128:(sub + 1) * 128], ident[:])
                        g = nt * 4 + sub
                        eng = nc.scalar if sub % 2 == 0 else nc.vector
                        sl = pT[:, sub * 128: sub * 128 + 64]
                        sh = pT[:, sub * 128 + 64: sub * 128 + 128]
                        if eng is nc.scalar:
                            nc.scalar.activation(vsb[:, g * 130: g * 130 + 64], sl, AF.Copy)
                            nc.scalar.activation(vsb[:, g * 130 + 65: g * 130 + 129], sh, AF.Copy)
                        else:
                            nc.vector.tensor_copy(vsb[:, g * 130: g * 130 + 64], sl)
                            nc.vector.tensor_copy(vsb[:, g * 130 + 65: g * 130 + 129], sh)
                    # q (input_ tokens only: last 1024 of each batch's 2048)
                    if nt % 4 >= 2:
                        pq = ps1.tile([128, 512], F32, tag="mm")
                        for kd in range(8):
                            nc.tensor.matmul(
                                pq[:], lhsT=wq[:, kd * 128:(kd + 1) * 128],
                                rhs=xtile[:, kd * 512:(kd + 1) * 512],
                                start=(kd == 0), stop=(kd == 7))
                        qc = (nt // 4) * 1024 + (nt % 4 - 2) * 512
                        nc.scalar.activation(qu[:, qc:qc + 512], pq[:], AF.Identity,
                                             bias=ub[:])
                        nc.scalar.activation(qv[:, qc:qc + 512], pq[:], AF.Identity,
                                             bias=vb[:])
                    # interleave R(0)+R(1) chunks once qv(b0)+pt are available
                    if step >= 2:
                        want = min(64, ((step - 1) * 64) // 12)
                        while r0_done < want:
                            r_chunk(r0_done // 32, r0_done % 32)
                            r0_done += 1
                while r0_done < 64:
                    r_chunk(r0_done // 32, r0_done % 32)
                    r0_done += 1
                flush_writes(slack=0)

            # ---------- phases 2+3: attention, R(bh+1) interleaved ----------
            PF = int(os.environ.get("K_PF", "3"))  # d1 prefetch depth (t-steps)
            with tc.tile_pool(name="ps2", bufs=4, space="PSUM") as ps2, \
                 tc.tile_pool(name="psav", bufs=2, space="PSUM") as psav, \
                 tc.tile_pool(name="d1p", bufs=int(os.environ.get("K_D1B", "8"))) as d1p, \
                 tc.tile_pool(name="attn", bufs=8) as atp, \
                 tc.tile_pool(name="nrm", bufs=2) as nrm:
                d1store = {bh_: [None] * 16 for bh_ in range(8)}

                def d1_load(bh_, t):
                    buf_, base_ = rslot(bh_)
                    d1 = d1p.tile([128, 1024], BF, tag="d1", name="d1")
                    q = nc.sync if (os.environ.get("K_D1Q", "1") == "0"
                                    or t % 2 == 0) else nc.scalar
                    q.dma_start_transpose(
                        d1[:],
                        _ap(buf_, base_ + 1023 + 128 * t, [[2048, 1024], [1, 128]]))
                    d1store[bh_][t] = d1

                for t in range(PF):
                    d1_load(0, t)
                for bh in range(8):
                    b, hl = bh // 2, bh % 2
                    hs = slice(hl * 64, (hl + 1) * 64)
                    d1s = d1store[bh]
                    ats = [None] * 16
                    pavs = [psav.tile([65, 512], F32, tag="av", name="pav")
                            for _ in range(2)]
                    rnext_done = 0
                    for t in range(16):
                        if t + PF < 16:
                            d1_load(bh, t + PF)
                        elif bh + 1 < 8:
                            d1_load(bh + 1, t + PF - 16)
                        g = b * 16 + t
                        ats[t] = []
                        scs = []
                        for i0h in range(2):
                            sc = ps2.tile([128, 512], F32, tag="sc", name="sc")
                            nc.tensor.matmul(
                                sc[:],
                                lhsT=kt[hs, b * T + t * 128: b * T + (t + 1) * 128],
                                rhs=qu[hs, b * S + i0h * 512: b * S + (i0h + 1) * 512],
                                start=True, stop=False)
                            scs.append(sc)
                        for i0h in range(2):
                            nc.tensor.matmul(
                                scs[i0h][:],
                                lhsT=ident[:],
                                rhs=d1s[t][:, i0h * 512:(i0h + 1) * 512],
                                start=False, stop=True)
                            at = atp.tile([128, 512], BF, tag="at", name="at")
                            nc.scalar.activation(at[:], scs[i0h][:], AF.Exp, scale=SCALE)
                            ats[t].append(at)
                        # AV for t-2 (software pipelined so PE never waits on exp)
                        if t > 1:
                            gav = b * 16 + t - 2
                            for i0h in range(2):
                                nc.tensor.matmul(
                                    pavs[i0h][:],
                                    lhsT=vsb[:, gav * 130 + hl * 65:
                                             gav * 130 + hl * 65 + 65],
                                    rhs=ats[t - 2][i0h][:],
                                    start=(t == 2), stop=False)
                            ats[t - 2] = None
                        # interleave R(bh+2): all 32 chunks across t-steps 0..11
                        if bh + 2 < 8:
                            want = min(32, (32 * (t + 1)) // int(os.environ.get("K_SPREAD", "11")))
                            while rnext_done < want:
                                r_chunk(bh + 2, rnext_done)
                                rnext_done += 1
                    flush_writes(slack=0)
                    for tl in (14, 15):
                        for i0h in range(2):
                            nc.tensor.matmul(
                                pavs[i0h][:],
                                lhsT=vsb[:, (b * 16 + tl) * 130 + hl * 65:
                                         (b * 16 + tl) * 130 + hl * 65 + 65],
                                rhs=ats[tl][i0h][:],
                                start=False, stop=(tl == 15))
                    # normalize: row 64 of psav = sum(exp); divide awv rows 0-63
                    awvt = awvt0 if hl == 0 else awvt1
                    for i0h in range(2):
                        awvu = nrm.tile([65, 512], BF, tag="awvu")
                        nc.scalar.activation(awvu[:], pavs[i0h][:], AF.Copy)
                        rec = nrm.tile([1, 512], F32, tag="rec")
                        nc.vector.reciprocal(rec[:], awvu[64:65, :])
                        recb = nrm.tile([64, 512], F32, tag="recb")
                        nc.gpsimd.partition_broadcast(recb[:], rec[:])
                        nc.vector.tensor_tensor(
                            out=awvt[:, b * S + i0h * 512: b * S + (i0h + 1) * 512],
                            in0=awvu[0:64, :], in1=recb[:], op=ALU.mult)
                    # stage this b-slice of awv^T into the A2A input buffer
                    a2a_in_h = a2a_in0_t if hl == 0 else a2a_in1_t
                    nc.sync.dma_start(
                        _ap(a2a_in_h, 2 * b * 32768,
                            [[512, 64], [32768, 2], [1, 512]]),
                        awvt[:, b * S: (b + 1) * S])
                    if bh == 6:
                        # awvt0 fully staged: overlap its AllToAll with attn(7)
                        if io.get("no_cc"):
                            nc.sync.dma_start(a2a_out0_t[:], a2a_in0_t[:])
                        else:
                            nc.gpsimd.collective_compute(
                                "AllToAll", ALU.bypass,
                                replica_groups=[list(range(NC))],
                                ins=[a2a_in0_t[:]], outs=[a2a_out0_t[:]],
                            )

        # ---- phase 4: 2-stage out-projection overlapping the A2As ----
        # stage 1: awvf rows 0:64 (heads hl=0, delivered by A2A#1) while A2A#2
        # is still in flight; stage 2 accumulates rows 64:128 after A2A#2.
        for c_ in range(8):
            nc.sync.dma_start(
                awvf[0:64, c_ * 512:(c_ + 1) * 512],
                _ap(a2a_out0_t, c_ * 32768, [[512, 64], [1, 512]]))
        if io.get("no_cc"):
            nc.sync.dma_start(a2a_out1_t[:], a2a_in1_t[:])
        else:
            nc.gpsimd.collective_compute(
                "AllToAll", ALU.bypass,
                replica_groups=[list(range(NC))],
                ins=[a2a_in1_t[:]], outs=[a2a_out1_t[:]],
            )
        nc.gpsimd.partition_broadcast(lng_b[:], lng_r[:])
        nc.gpsimd.partition_broadcast(lnb_b[:], lnb_r[:])

        with tc.tile_pool(name="outp", bufs=2) as op_, \
             tc.tile_pool(name="stat", bufs=2) as stp, \
             tc.tile_pool(name="ps3", bufs=8, space="PSUM") as ps3:
            pos = []
            resids = []
            for tt in range(4):
                resid = op_.tile([128, D], F32, tag="resid", name="resid")
                nc.sync.dma_start(resid[:], io["resid"][tt * 128:(tt + 1) * 128, :])
                resids.append(resid)
                for n2 in range(2):
                    po = ps3.tile([128, 512], F32, tag="mm", name="po")
                    for kd in range(8):
                        nc.tensor.matmul(
                            po[:],
                            lhsT=awvf[0:64,
                                      kd * 512 + tt * 128: kd * 512 + (tt + 1) * 128],
                            rhs=wout[0:64, kd * D + n2 * 512: kd * D + n2 * 512 + 512],
                            start=(kd == 0), stop=False)
                    pos.append(po)
            for c_ in range(8):
                nc.sync.dma_start(
                    awvf[64:128, c_ * 512:(c_ + 1) * 512],
                    _ap(a2a_out1_t, c_ * 32768, [[512, 64], [1, 512]]))
            for tt in range(4):
                resid = resids[tt]
                o = op_.tile([128, D], F32, tag="o")
                for n2 in range(2):
                    po = pos[tt * 2 + n2]
                    for kd in range(8):
                        nc.tensor.matmul(
                            po[:],
                            lhsT=awvf[64:128,
                                      kd * 512 + tt * 128: kd * 512 + (tt + 1) * 128],
                            rhs=wout[64:128, kd * D + n2 * 512: kd * D + n2 * 512 + 512],
                            start=False, stop=(kd == 7))
                    nc.vector.tensor_add(
                        o[:, n2 * 512:(n2 + 1) * 512], po[:],
                        resid[:, n2 * 512:(n2 + 1) * 512])
                # LayerNorm over D
                sm = stp.tile([128, 1], F32, tag="sm")
                nc.vector.tensor_reduce(sm[:], o[:], axis=mybir.AxisListType.X,
                                        op=ALU.add)
                mean = stp.tile([128, 1], F32, tag="mean")
                nc.vector.tensor_scalar_mul(mean[:], sm[:], 1.0 / D)
                cent = op_.tile([128, D], F32, tag="cent")
                nc.vector.tensor_scalar(out=cent[:], in0=o[:], scalar1=mean[:],
                                        scalar2=None, op0=ALU.subtract)
                sq = op_.tile([128, D], F32, tag="sq")
                ssq = stp.tile([128, 1], F32, tag="ssq")
                nc.scalar.activation(sq[:], cent[:], AF.Square, accum_out=ssq[:])
                veps = stp.tile([128, 1], F32, tag="veps")
                nc.vector.tensor_scalar(out=veps[:], in0=ssq[:], scalar1=1.0 / D,
                                        scalar2=LN_EPS, op0=ALU.mult, op1=ALU.add)
                std = stp.tile([128, 1], F32, tag="std")
                nc.scalar.activation(std[:], veps[:], AF.Sqrt)
                rstd = stp.tile([128, 1], F32, tag="rstd")
                nc.vector.reciprocal(rstd[:], std[:])
                y1 = op_.tile([128, D], F32, tag="o")
                nc.vector.scalar_tensor_tensor(
                    out=y1[:], in0=cent[:], scalar=rstd[:], in1=lng_b[:],
                    op0=ALU.mult, op1=ALU.mult)
                yf = op_.tile([128, D], F32, tag="cent")
                nc.vector.tensor_add(yf[:], y1[:], lnb_b[:])
                nc.sync.dma_start(out_t[tt * 128:(tt + 1) * 128, :], yf[:])


_compiled = None


def _build(no_cc=False):
    nc = bacc.Bacc("TRN2", target_bir_lowering=False, debug=False, num_devices=NC)
    io = {}
    io["xt"] = nc.dram_tensor("xt", [D, B * T], BF, kind="ExternalInput")
    io["pt"] = nc.dram_tensor("pt", [D, T], BF, kind="ExternalInput")
    io["wk"] = nc.dram_tensor("wk", [128, D], BF, kind="ExternalInput")
    io["wv"] = nc.dram_tensor("wv", [128, D], BF, kind="ExternalInput")
    io["wq"] = nc.dram_tensor("wq", [128, D], BF, kind="ExternalInput")
    io["wp"] = nc.dram_tensor("wp", [128, D], BF, kind="ExternalInput")
    io["wout"] = nc.dram_tensor("wout", [H * d, D], BF, kind="ExternalInput")
    io["ub"] = nc.dram_tensor("ub", [128, 1], F32, kind="ExternalInput").ap()
    io["vb"] = nc.dram_tensor("vb", [128, 1], F32, kind="ExternalInput").ap()
    io["lng"] = nc.dram_tensor("lng", [1, D], F32, kind="ExternalInput").ap()
    io["lnb"] = nc.dram_tensor("lnb", [1, D], F32, kind="ExternalInput").ap()
    io["resid"] = nc.dram_tensor("resid", [512, D], F32, kind="ExternalInput").ap()
    io["out"] = nc.dram_tensor("out", [512, D], F32, kind="ExternalOutput").ap()
    for nm in ("r2a", "r2b", "r2c", "r2d"):
        io[nm] = nc.dram_tensor(nm, [2, RSZ], BF)
    for nm in ("a2a_in0", "a2a_in1", "a2a_out0", "a2a_out1"):
        io[nm] = nc.dram_tensor(nm, [NC, 64, 512], BF)
    io["no_cc"] = no_cc
    with tile.TileContext(nc) as tc:
        _body(nc, tc, io)
    nc.compile()
    return nc


def _shard(inputs):
    x = np.asarray(inputs["input_"], np.float32)
    pos = np.asarray(inputs["pos_embs"], np.float32)
    mem = np.asarray(inputs["memory"], np.float32)
    u = np.asarray(inputs["u"], np.float32).reshape(-1)
    v = np.asarray(inputs["v"], np.float32).reshape(-1)
    W_kv = np.asarray(inputs["W_kv"], np.float32)
    W_q = np.asarray(inputs["W_q"], np.float32)
    W_p = np.asarray(inputs["W_p"], np.float32)
    W_out = np.asarray(inputs["W_out"], np.float32)
    lng = np.asarray(inputs["ln_g"], np.float32).reshape(1, D)
    lnb = np.asarray(inputs["ln_b"], np.float32).reshape(1, D)

    x_mem = np.concatenate([mem, x], axis=0)                  # (T, B, D)
    xt = np.ascontiguousarray(
        x_mem.transpose(2, 1, 0).reshape(D, B * T)            # (D, b-major tokens)
    ).astype(bf16)
    pt = np.ascontiguousarray(pos.T).astype(bf16)             # (D, T)
    wout_b = W_out.astype(bf16)

    in_maps = []
    for c in range(NC):
        hs = slice(c * 128, (c + 1) * 128)
        b, i0 = c // 2, (c % 2) * 512
        in_maps.append({
            "xt": xt,
            "pt": pt,
            "wk": _wswiz(W_kv[:, hs]),
            "wv": _wswiz(W_kv[:, H * d + c * 128: H * d + (c + 1) * 128]),
            "wq": _wswiz(W_q[:, hs]),
            "wp": _wswiz(W_p[:, hs]),
            "wout": wout_b,
            "ub": np.ascontiguousarray(u[hs].reshape(128, 1)),
            "vb": np.ascontiguousarray(v[hs].reshape(128, 1)),
            "lng": lng,
            "lnb": lnb,
            "resid": np.ascontiguousarray(x[i0:i0 + 512, b, :]),
        })
    return in_maps


def _wswiz(w):
    """(1024, 128) -> (128, 1024) so wt_sb[p, kd*128+j] = w[p+128*kd, j] loads
    as 128 contiguous 2 KiB rows."""
    return np.ascontiguousarray(
        w.reshape(8, 128, 128).transpose(1, 0, 2).reshape(128, 1024)).astype(bf16)


LAST_RESULTS = None


def kernel(**inputs):
    global _compiled, LAST_RESULTS
    if _compiled is None:
        _compiled = _build()
    nc = _compiled
    in_maps = _shard(inputs)
    res = run_bass_kernel_spmd(nc, in_maps, core_ids=list(range(NC)))
    LAST_RESULTS = res
    out = np.empty((S, B, D), np.float32)
    for c in range(NC):
        b, i0 = c // 2, (c % 2) * 512
        out[i0:i0 + 512, b, :] = res.results[c]["out"]
    return out


# revision 50
# speedup vs baseline: 11.2937x; 5.7812x over previous
"""Transformer-XL relative-position multi-head attention on 8 Trainium2 cores.

Sharding: tensor-parallel over heads (2 heads/core) for projections+attention,
then AllToAll to redistribute awv^T head-major -> token-sharded, out-projection
+ residual + LayerNorm token-sharded (512 tokens/core).

Rel-shift: R = (q+v) @ p^T is computed in (i, r) coords and bounced through
DRAM with row stride 2049, appending a zero element per row:
    R'[i*2049 + r] = R[i, r],  R'[i*2049 + 2048] = 0
A single transposed read at partition stride 2048 then reproduces the exact
reference rel-shift semantics for every (i, j):
    shifted[i, j] = R'[2048*i + 1023 + j]
      = R[i, 1023+j-i]        (j-i <= 1024)
      = 0                     (j-i == 1025, the appended zero)
      = R[i+1, j-i-1026]      (j-i >= 1026, the reference reshape wrap)
so scores need no masks/adds: content matmul accumulates + one injection
matmul of the transposed tile per 512-column PSUM bank.

Scores live in (key j, query i) layout so AV needs no transposes: exp is
unnormalized, a ones-column in the AV matmul produces the softmax denominator,
and the division is applied to awv^T (65 x 512 per head) after the fact.

R(bh+1) compute/copy/write is software-pipelined into attn(bh)'s t-loop, and
R(0) into the phase-1 projection loop, to keep PE continuously busy.
"""
import os
import numpy as np
import ml_dtypes

import concourse.bass as bass
import concourse.mybir as mybir
import concourse.tile as tile
from concourse import bacc
from concourse.bass_utils import run_bass_kernel_spmd
from concourse.masks import make_identity
import bass_rust

BF = mybir.dt.bfloat16
F32 = mybir.dt.float32
AF = mybir.ActivationFunctionType
ALU = mybir.AluOpType
bf16 = ml_dtypes.bfloat16

S = 1024
PREV = 1024
T = 2048
B = 4
D = 1024
H = 16
d = 64
NC = 8
SCALE = 1.0 / 8.0
LN_EPS = 1e-5

RROW = 2049                      # R' row stride (2048 scores + 1 zero)
RSZ = 1023 * RROW + 2048 + 1     # per-bh R' slot size = 2098176


def _ap(handle, offset, pattern):
    return bass_rust.AP(tensor=handle, offset=offset, ap=pattern)


def _body(nc, tc, io):
    out_t = io["out"]
    r2 = [io["r2a"], io["r2b"], io["r2c"], io["r2d"]]  # rotation: no false deps
    a2a_in0_t = io["a2a_in0"]
    a2a_in1_t = io["a2a_in1"]
    a2a_out0_t = io["a2a_out0"]
    a2a_out1_t = io["a2a_out1"]

    def rslot(bh):
        return r2[bh % 4], (bh // 4) * RSZ

    with tc.tile_pool(name="res", bufs=1) as res:
        # ---- persistent tiles ----
        kt = res.tile([128, B * T], BF, tag="kt")          # k^T, (2*d, b-major tokens)
        vsb = res.tile([128, 64 * 130], BF, tag="vsb")     # [v_h0|1|v_h1|1] per j-tile
        qu = res.tile([128, B * S], BF, tag="qu")
        qv = res.tile([128, B * S], BF, tag="qv")
        pt = res.tile([128, T], BF, tag="pt")
        wout = res.tile([128, 8 * D], BF, tag="wout")
        wk = res.tile([128, D], BF, tag="wk")
        wv = res.tile([128, D], BF, tag="wv")
        wq = res.tile([128, D], BF, tag="wq")
        wp = res.tile([128, D], BF, tag="wp")
        awvt0 = res.tile([64, B * S], BF, tag="awvt0")
        awvt1 = res.tile([64, B * S], BF, tag="awvt1")
        awvf = res.tile([128, 8 * 512], BF, tag="awvf")    # gathered awv^T K-tiles
        ident = res.tile([128, 128], BF, tag="ident")
        zcol = res.tile([128, 8], BF, tag="zcol")          # 1024 zeros for R' pad col
        ub = res.tile([128, 1], F32, tag="ub")
        vb = res.tile([128, 1], F32, tag="vb")
        lng_r = res.tile([1, D], F32, tag="lngr")
        lnb_r = res.tile([1, D], F32, tag="lnbr")
        lng_b = res.tile([128, D], F32, tag="lngb")
        lnb_b = res.tile([128, D], F32, tag="lnbb")

        # ---- constant loads ----
        # wp first: the p-projection is the first PE work
        nc.sync.dma_start(wp[:], _ap(io["wp"], 0, [[1024, 128], [1, 1024]]))
        make_identity(nc, ident[:])
        nc.gpsimd.memset(vsb[:], 1.0)
        nc.gpsimd.memset(zcol[:], 0.0)

        with tc.tile_pool(name="psr", bufs=2, space="PSUM") as psr, \
             tc.tile_pool(name="rsp", bufs=int(os.environ.get("K_RSP", "6"))) as rsp:

            # ---------- R(bh) chunk machinery (32 chunks per bh) ----------
            rstate = {}
            pending_writes = []  # (emit_after_chunk_count, closure)
            chunk_clock = [0]

            def flush_writes(slack=int(os.environ.get("K_SLACK", "10"))):
                """Emit deferred R' writes whose source copies finished >= slack
                chunk-slots ago, so the sync queue never head-of-line blocks."""
                while pending_writes and \
                        pending_writes[0][0] + slack <= chunk_clock[0]:
                    pending_writes.pop(0)[1]()

            def r_chunk(bh, c):
                """Emit chunk c (it=c//4, rt=c%4) of R'(bh)."""
                b_, hl_ = bh // 2, bh % 2
                hs_ = slice(hl_ * 64, (hl_ + 1) * 64)
                it, rt = c // 4, c % 4
                if rt == 0:
                    rstate["rs"] = rsp.tile([128, T], BF, tag="rs", name="rs")
                rs = rstate["rs"]
                pr = psr.tile([128, 512], F32, tag="rc")
                nc.tensor.matmul(
                    pr[:],
                    lhsT=qv[hs_, b_ * S + it * 128: b_ * S + (it + 1) * 128],
                    rhs=pt[hs_, rt * 512:(rt + 1) * 512],
                    start=True, stop=True)
                if rt == 0:
                    nc.scalar.activation(rs[:, rt * 512:(rt + 1) * 512], pr[:], AF.Copy)
                else:
                    nc.vector.tensor_copy(rs[:, rt * 512:(rt + 1) * 512], pr[:])
                chunk_clock[0] += 1
                if rt == 3:
                    buf, base = rslot(bh)
                    off = base + it * 128 * RROW

                    def wr(rs=rs, buf=buf, off=off):
                        nc.sync.dma_start(
                            _ap(buf, off, [[RROW, 128], [1, 1024]]),
                            rs[:, 0:1024])
                        nc.sync.dma_start(
                            _ap(buf, off + 1024, [[RROW, 128], [1, 1024]]),
                            rs[:, 1024:2048])
                    pending_writes.append((chunk_clock[0], wr))
                flush_writes()

            # ---------- phase 1: projections (R(0) interleaved) ----------
            with tc.tile_pool(name="xt", bufs=3) as xtp, \
                 tc.tile_pool(name="ps1", bufs=2, space="PSUM") as ps1, \
                 tc.tile_pool(name="psv", bufs=2, space="PSUM") as psv, \
                 tc.tile_pool(name="psT", bufs=2, space="PSUM") as psT, \
                 tc.tile_pool(name="vts", bufs=2) as vtsp:
                # p^T first (R(0) needs it); all 4 loads issued up front
                ptiles = []
                for rt in range(4):
                    ptile = xtp.tile([128, 8 * 512], BF, tag="pt", name="ptile")
                    nc.sync.dma_start(
                        ptile[:],
                        _ap(io["pt"], rt * 512, [[T, 128], [T * 128, 8], [1, 512]]))
                    ptiles.append(ptile)
                for wt_sb, wt_h in ((wk, io["wk"]), (wv, io["wv"]), (wq, io["wq"])):
                    nc.sync.dma_start(
                        wt_sb[:], _ap(wt_h, 0, [[1024, 128], [1, 1024]]))
                nc.sync.dma_start(ub[:], io["ub"][:])
                nc.sync.dma_start(vb[:], io["vb"][:])
                xpre = {}
                for nt in (2, 3):
                    xtile = xtp.tile([128, 8 * 512], BF, tag="xt", name="xt")
                    nc.gpsimd.dma_start(
                        xtile[:],
                        _ap(io["xt"], nt * 512,
                            [[B * T, 128], [B * T * 128, 8], [1, 512]]))
                    xpre[nt] = xtile
                for rt in range(4):
                    ptile = ptiles[rt]
                    pp = ps1.tile([128, 512], F32, tag="mm")
                    for kd in range(8):
                        nc.tensor.matmul(
                            pp[:], lhsT=wp[:, kd * 128:(kd + 1) * 128],
                            rhs=ptile[:, kd * 512:(kd + 1) * 512],
                            start=(kd == 0), stop=(kd == 7))
                    nc.vector.tensor_copy(pt[:, rt * 512:(rt + 1) * 512], pp[:])

                # deferred constant loads (needed later than the projections)
                for buf in r2:
                    for s_ in range(2):
                        nc.sync.dma_start(
                            _ap(buf, s_ * RSZ + 2048, [[RROW, 1024], [1, 1]]),
                            zcol[:])
                nc.sync.dma_start(
                    wout[:], _ap(io["wout"], 0, [[1024, 128], [131072, 8], [1, 1024]]))
                nc.sync.dma_start(lng_r[:], io["lng"][:])
                nc.sync.dma_start(lnb_r[:], io["lnb"][:])

                # token slices; b0's q-slices first so R(0)/R(1) start early
                nt_order = [2, 3, 0, 1] + list(range(4, 16))
                r0_done = 0  # chunks of R(0) followed by R(1): 64 total
                for step, nt in enumerate(nt_order):
                    if nt in xpre:
                        xtile = xpre.pop(nt)
                    else:
                        xtile = xtp.tile([128, 8 * 512], BF, tag="xt", name="xt")
                        nc.gpsimd.dma_start(
                            xtile[:],
                            _ap(io["xt"], nt * 512,
                                [[B * T, 128], [B * T * 128, 8], [1, 512]]))
                    # k^T
                    ps = ps1.tile([128, 512], F32, tag="mm")
                    for kd in range(8):
                        nc.tensor.matmul(
                            ps[:], lhsT=wk[:, kd * 128:(kd + 1) * 128],
                            rhs=xtile[:, kd * 512:(kd + 1) * 512],
                            start=(kd == 0), stop=(kd == 7))
                    nc.scalar.activation(kt[:, nt * 512:(nt + 1) * 512], ps[:], AF.Copy)
                    # v^T then PE-transpose to token-partition layout
                    pv = psv.tile([128, 512], F32, tag="v")
                    for kd in range(8):
                        nc.tensor.matmul(
                            pv[:], lhsT=wv[:, kd * 128:(kd + 1) * 128],
                            rhs=xtile[:, kd * 512:(kd + 1) * 512],
                            start=(kd == 0), stop=(kd == 7))
                    vt = vtsp.tile([128, 512], BF, tag="vts")
                    nc.vector.tensor_copy(vt[:], pv[:])
                    pT = psT.tile([128, 512], BF, tag="T")
                    for sub in range(4):
                        nc.tensor.transpose(
                            pT[:, sub * 128:(sub + 1) * 128],
                            vt[:, sub * 128:(sub + 1) * 128], ident[:])
                        g = nt * 4 + sub
                        eng = nc.scalar if sub % 2 == 0 else nc.vector
                        sl = pT[:, sub * 128: sub * 128 + 64]
                        sh = pT[:, sub * 128 + 64: sub * 128 + 128]
                        if eng is nc.scalar:
                            nc.scalar.activation(vsb[:, g * 130: g * 130 + 64], sl, AF.Copy)
                            nc.scalar.activation(vsb[:, g * 130 + 65: g * 130 + 129], sh, AF.Copy)
                        else:
                            nc.vector.tensor_copy(vsb[:, g * 130: g * 130 + 64], sl)
                            nc.vector.tensor_copy(vsb[:, g * 130 + 65: g * 130 + 129], sh)
                    # q (input_ tokens only: last 1024 of each batch's 2048)
                    if nt % 4 >= 2:
                        pq = ps1.tile([128, 512], F32, tag="mm")
                        for kd in range(8):
                            nc.tensor.matmul(
                                pq[:], lhsT=wq[:, kd * 128:(kd + 1) * 128],
                                rhs=xtile[:, kd * 512:(kd + 1) * 512],
                                start=(kd == 0), stop=(kd == 7))
                        qc = (nt // 4) * 1024 + (nt % 4 - 2) * 512
                        nc.scalar.activation(qu[:, qc:qc + 512], pq[:], AF.Identity,
                                             bias=ub[:])
                        nc.scalar.activation(qv[:, qc:qc + 512], pq[:], AF.Identity,
                                             bias=vb[:])
                    # interleave R(0)+R(1) chunks once qv(b0)+pt are available
                    if step >= 2:
                        want = min(64, ((step - 1) * 64) // 12)
                        while r0_done < want:
                            r_chunk(r0_done // 32, r0_done % 32)
                            r0_done += 1
                while r0_done < 64:
                    r_chunk(r0_done // 32, r0_done % 32)
                    r0_done += 1
                flush_writes(slack=0)

            # ---------- phases 2+3: attention, R(bh+1) interleaved ----------
            PF = int(os.environ.get("K_PF", "3"))  # d1 prefetch depth (t-steps)
            with tc.tile_pool(name="ps2", bufs=4, space="PSUM") as ps2, \
                 tc.tile_pool(name="psav", bufs=2, space="PSUM") as psav, \
                 tc.tile_pool(name="d1p", bufs=int(os.environ.get("K_D1B", "8"))) as d1p, \
                 tc.tile_pool(name="attn", bufs=int(os.environ.get("K_ATP", "8"))) as atp, \
                 tc.tile_pool(name="nrm", bufs=2) as nrm:
                d1store = {bh_: [None] * 16 for bh_ in range(8)}

                def d1_load(bh_, t):
                    buf_, base_ = rslot(bh_)
                    d1 = d1p.tile([128, 1024], BF, tag="d1", name="d1")
                    q = nc.sync if (os.environ.get("K_D1Q", "1") == "0"
                                    or t % 2 == 0) else nc.scalar
                    q.dma_start_transpose(
                        d1[:],
                        _ap(buf_, base_ + 1023 + 128 * t, [[2048, 1024], [1, 128]]))
                    d1store[bh_][t] = d1

                for t in range(PF):
                    d1_load(0, t)
                for bh in range(8):
                    b, hl = bh // 2, bh % 2
                    hs = slice(hl * 64, (hl + 1) * 64)
                    d1s = d1store[bh]
                    ats = [None] * 16
                    pavs = [psav.tile([65, 512], F32, tag="av", name="pav")
                            for _ in range(2)]
                    rnext_done = 0
                    for t in range(16):
                        if t + PF < 16:
                            d1_load(bh, t + PF)
                        elif bh + 1 < 8:
                            d1_load(bh + 1, t + PF - 16)
                        g = b * 16 + t
                        ats[t] = []
                        scs = []
                        for i0h in range(2):
                            sc = ps2.tile([128, 512], F32, tag="sc", name="sc")
                            nc.tensor.matmul(
                                sc[:],
                                lhsT=kt[hs, b * T + t * 128: b * T + (t + 1) * 128],
                                rhs=qu[hs, b * S + i0h * 512: b * S + (i0h + 1) * 512],
                                start=True, stop=False)
                            scs.append(sc)
                        for i0h in range(2):
                            nc.tensor.matmul(
                                scs[i0h][:],
                                lhsT=ident[:],
                                rhs=d1s[t][:, i0h * 512:(i0h + 1) * 512],
                                start=False, stop=True)
                            at = atp.tile([128, 512], BF, tag="at", name="at")
                            nc.scalar.activation(at[:], scs[i0h][:], AF.Exp, scale=SCALE)
                            ats[t].append(at)
                        # AV for t-2 (software pipelined so PE never waits on exp)
                        if t > 1:
                            gav = b * 16 + t - 2
                            for i0h in range(2):
                                nc.tensor.matmul(
                                    pavs[i0h][:],
                                    lhsT=vsb[:, gav * 130 + hl * 65:
                                             gav * 130 + hl * 65 + 65],
                                    rhs=ats[t - 2][i0h][:],
                                    start=(t == 2), stop=False)
                            ats[t - 2] = None
                        # interleave R(bh+2): all 32 chunks across t-steps 0..11
                        if bh + 2 < 8:
                            want = min(32, (32 * (t + 1)) // int(os.environ.get("K_SPREAD", "11")))
                            while rnext_done < want:
                                r_chunk(bh + 2, rnext_done)
                                rnext_done += 1
                    flush_writes(slack=0)
                    for tl in (14, 15):
                        for i0h in range(2):
                            nc.tensor.matmul(
                                pavs[i0h][:],
                                lhsT=vsb[:, (b * 16 + tl) * 130 + hl * 65:
                                         (b * 16 + tl) * 130 + hl * 65 + 65],
                                rhs=ats[tl][i0h][:],
                                start=False, stop=(tl == 15))
                    # normalize: row 64 of psav = sum(exp); divide awv rows 0-63
                    awvt = awvt0 if hl == 0 else awvt1
                    for i0h in range(2):
                        awvu = nrm.tile([65, 512], BF, tag="awvu")
                        nc.scalar.activation(awvu[:], pavs[i0h][:], AF.Copy)
                        rec = nrm.tile([1, 512], F32, tag="rec")
                        nc.vector.reciprocal(rec[:], awvu[64:65, :])
                        recb = nrm.tile([64, 512], F32, tag="recb")
                        nc.gpsimd.partition_broadcast(recb[:], rec[:])
                        nc.vector.tensor_tensor(
                            out=awvt[:, b * S + i0h * 512: b * S + (i0h + 1) * 512],
                            in0=awvu[0:64, :], in1=recb[:], op=ALU.mult)
                    # stage this b-slice of awv^T into the A2A input buffer
                    a2a_in_h = a2a_in0_t if hl == 0 else a2a_in1_t
                    nc.sync.dma_start(
                        _ap(a2a_in_h, 2 * b * 32768,
                            [[512, 64], [32768, 2], [1, 512]]),
                        awvt[:, b * S: (b + 1) * S])
                    if bh == 6:
                        # awvt0 fully staged: overlap its AllToAll with attn(7)
                        if io.get("no_cc"):
                            nc.sync.dma_start(a2a_out0_t[:], a2a_in0_t[:])
                        else:
                            nc.gpsimd.collective_compute(
                                "AllToAll", ALU.bypass,
                                replica_groups=[list(range(NC))],
                                ins=[a2a_in0_t[:]], outs=[a2a_out0_t[:]],
                            )

        # ---- phase 4: 2-stage out-projection overlapping the A2As ----
        # stage 1: awvf rows 0:64 (heads hl=0, delivered by A2A#1) while A2A#2
        # is still in flight; stage 2 accumulates rows 64:128 after A2A#2.
        for c_ in range(8):
            nc.sync.dma_start(
                awvf[0:64, c_ * 512:(c_ + 1) * 512],
                _ap(a2a_out0_t, c_ * 32768, [[512, 64], [1, 512]]))
        if io.get("no_cc"):
            nc.sync.dma_start(a2a_out1_t[:], a2a_in1_t[:])
        else:
            nc.gpsimd.collective_compute(
                "AllToAll", ALU.bypass,
                replica_groups=[list(range(NC))],
                ins=[a2a_in1_t[:]], outs=[a2a_out1_t[:]],
            )
        nc.gpsimd.partition_broadcast(lng_b[:], lng_r[:])
        nc.gpsimd.partition_broadcast(lnb_b[:], lnb_r[:])

        with tc.tile_pool(name="outp", bufs=2) as op_, \
             tc.tile_pool(name="stat", bufs=2) as stp, \
             tc.tile_pool(name="ps3", bufs=8, space="PSUM") as ps3:
            pos = []
            resids = []
            for tt in range(4):
                resid = op_.tile([128, D], F32, tag="resid", name="resid")
                nc.sync.dma_start(resid[:], io["resid"][tt * 128:(tt + 1) * 128, :])
                resids.append(resid)
                for n2 in range(2):
                    po = ps3.tile([128, 512], F32, tag="mm", name="po")
                    for kd in range(8):
                        nc.tensor.matmul(
                            po[:],
                            lhsT=awvf[0:64,
                                      kd * 512 + tt * 128: kd * 512 + (tt + 1) * 128],
                            rhs=wout[0:64, kd * D + n2 * 512: kd * D + n2 * 512 + 512],
                            start=(kd == 0), stop=False)
                    pos.append(po)
            for c_ in range(8):
                nc.sync.dma_start(
                    awvf[64:128, c_ * 512:(c_ + 1) * 512],
                    _ap(a2a_out1_t, c_ * 32768, [[512, 64], [1, 512]]))
            for tt in range(4):
                resid = resids[tt]
                o = op_.tile([128, D], F32, tag="o")
                for n2 in range(2):
                    po = pos[tt * 2 + n2]
                    for kd in range(8):
                        nc.tensor.matmul(
                            po[:],
                            lhsT=awvf[64:128,
                                      kd * 512 + tt * 128: kd * 512 + (tt + 1) * 128],
                            rhs=wout[64:128, kd * D + n2 * 512: kd * D + n2 * 512 + 512],
                            start=False, stop=(kd == 7))
                    nc.vector.tensor_add(
                        o[:, n2 * 512:(n2 + 1) * 512], po[:],
                        resid[:, n2 * 512:(n2 + 1) * 512])
                # LayerNorm over D
                sm = stp.tile([128, 1], F32, tag="sm")
                nc.vector.tensor_reduce(sm[:], o[:], axis=mybir.AxisListType.X,
                                        op=ALU.add)
                mean = stp.tile([128, 1], F32, tag="mean")
                nc.vector.tensor_scalar_mul(mean[:], sm[:], 1.0 / D)
                cent = op_.tile([128, D], F32, tag="cent")
                nc.vector.tensor_scalar(out=cent[:], in0=o[:], scalar1=mean[:],
                                        scalar2=None, op0=ALU.subtract)
                sq = op_.tile([128, D], F32, tag="sq")
                ssq = stp.tile([128, 1], F32, tag="ssq")
                nc.scalar.activation(sq[:], cent[:], AF.Square, accum_out=ssq[:])
                veps = stp.tile([128, 1], F32, tag="veps")
                nc.vector.tensor_scalar(out=veps[:], in0=ssq[:], scalar1=1.0 / D,
                                        scalar2=LN_EPS, op0=ALU.mult, op1=ALU.add)
                std = stp.tile([128, 1], F32, tag="std")
                nc.scalar.activation(std[:], veps[:], AF.Sqrt)
                rstd = stp.tile([128, 1], F32, tag="rstd")
                nc.vector.reciprocal(rstd[:], std[:])
                y1 = op_.tile([128, D], F32, tag="o")
                nc.vector.scalar_tensor_tensor(
                    out=y1[:], in0=cent[:], scalar=rstd[:], in1=lng_b[:],
                    op0=ALU.mult, op1=ALU.mult)
                yf = op_.tile([128, D], F32, tag="cent")
                nc.vector.tensor_add(yf[:], y1[:], lnb_b[:])
                nc.sync.dma_start(out_t[tt * 128:(tt + 1) * 128, :], yf[:])


_compiled = None


def _build(no_cc=False):
    nc = bacc.Bacc("TRN2", target_bir_lowering=False, debug=False, num_devices=NC)
    io = {}
    io["xt"] = nc.dram_tensor("xt", [D, B * T], BF, kind="ExternalInput")
    io["pt"] = nc.dram_tensor("pt", [D, T], BF, kind="ExternalInput")
    io["wk"] = nc.dram_tensor("wk", [128, D], BF, kind="ExternalInput")
    io["wv"] = nc.dram_tensor("wv", [128, D], BF, kind="ExternalInput")
    io["wq"] = nc.dram_tensor("wq", [128, D], BF, kind="ExternalInput")
    io["wp"] = nc.dram_tensor("wp", [128, D], BF, kind="ExternalInput")
    io["wout"] = nc.dram_tensor("wout", [H * d, D], BF, kind="ExternalInput")
    io["ub"] = nc.dram_tensor("ub", [128, 1], F32, kind="ExternalInput").ap()
    io["vb"] = nc.dram_tensor("vb", [128, 1], F32, kind="ExternalInput").ap()
    io["lng"] = nc.dram_tensor("lng", [1, D], F32, kind="ExternalInput").ap()
    io["lnb"] = nc.dram_tensor("lnb", [1, D], F32, kind="ExternalInput").ap()
    io["resid"] = nc.dram_tensor("resid", [512, D], F32, kind="ExternalInput").ap()
    io["out"] = nc.dram_tensor("out", [512, D], F32, kind="ExternalOutput").ap()
    for nm in ("r2a", "r2b", "r2c", "r2d"):
        io[nm] = nc.dram_tensor(nm, [2, RSZ], BF)
    for nm in ("a2a_in0", "a2a_in1", "a2a_out0", "a2a_out1"):
        io[nm] = nc.dram_tensor(nm, [NC, 64, 512], BF)
    io["no_cc"] = no_cc
    with tile.TileContext(nc) as tc:
        _body(nc, tc, io)
    nc.compile()
    return nc


def _shard(inputs):
    x = np.asarray(inputs["input_"], np.float32)
    pos = np.asarray(inputs["pos_embs"], np.float32)
    mem = np.asarray(inputs["memory"], np.float32)
    u = np.asarray(inputs["u"], np.float32).reshape(-1)
    v = np.asarray(inputs["v"], np.float32).reshape(-1)
    W_kv = np.asarray(inputs["W_kv"], np.float32)
    W_q = np.asarray(inputs["W_q"], np.float32)
    W_p = np.asarray(inputs["W_p"], np.float32)
    W_out = np.asarray(inputs["W_out"], np.float32)
    lng = np.asarray(inputs["ln_g"], np.float32).reshape(1, D)
    lnb = np.asarray(inputs["ln_b"], np.float32).reshape(1, D)

    x_mem = np.concatenate([mem, x], axis=0)                  # (T, B, D)
    xt = np.ascontiguousarray(
        x_mem.transpose(2, 1, 0).reshape(D, B * T)            # (D, b-major tokens)
    ).astype(bf16)
    pt = np.ascontiguousarray(pos.T).astype(bf16)             # (D, T)
    wout_b = W_out.astype(bf16)

    in_maps = []
    for c in range(NC):
        hs = slice(c * 128, (c + 1) * 128)
        b, i0 = c // 2, (c % 2) * 512
        in_maps.append({
            "xt": xt,
            "pt": pt,
            "wk": _wswiz(W_kv[:, hs]),
            "wv": _wswiz(W_kv[:, H * d + c * 128: H * d + (c + 1) * 128]),
            "wq": _wswiz(W_q[:, hs]),
            "wp": _wswiz(W_p[:, hs]),
            "wout": wout_b,
            "ub": np.ascontiguousarray(u[hs].reshape(128, 1)),
            "vb": np.ascontiguousarray(v[hs].reshape(128, 1)),
            "lng": lng,
            "lnb": lnb,
            "resid": np.ascontiguousarray(x[i0:i0 + 512, b, :]),
        })
    return in_maps


def _wswiz(w):
    """(1024, 128) -> (128, 1024) so wt_sb[p, kd*128+j] = w[p+128*kd, j] loads
    as 128 contiguous 2 KiB rows."""
    return np.ascontiguousarray(
        w.reshape(8, 128, 128).transpose(1, 0, 2).reshape(128, 1024)).astype(bf16)


LAST_RESULTS = None


def kernel(**inputs):
    global _compiled, LAST_RESULTS
    if _compiled is None:
        _compiled = _build()
    nc = _compiled
    in_maps = _shard(inputs)
    res = run_bass_kernel_spmd(nc, in_maps, core_ids=list(range(NC)))
    LAST_RESULTS = res
    out = np.empty((S, B, D), np.float32)
    for c in range(NC):
        b, i0 = c // 2, (c % 2) * 512
        out[i0:i0 + 512, b, :] = res.results[c]["out"]
    return out
